# revision 1
# baseline (speedup 1.0000x reference)
"""Fused dual-softmax attention (nn_Attention sparse_attention) on 8x TRN2.

Sharding: data-parallel over batch -- one batch element per NeuronCore.

Per-core pipeline (feature-major activations, key-major score matrices).
All matmul operands are fp16 (1 cyc/row on PE, fast weight load); PSUM
accumulation is fp32; softmax denominators and reciprocals are fp32.

  qT/kT = w_{q,k} @ x.T            (w0*SCALE pre-folded into wq)
  v     = x @ w_v.T  token-major, augmented with a ones column per head
  per head h:
    lsim   = lid_h.T @ lid_h       (symmetric; sqrt(SCALE) folded into lidT)
    explid = exp(lsim - 8)         (ACT; bias keeps fp16 range; accum -> sl
                                    row sums via symmetry; shift cancels in
                                    the lidar softmax)
    bc     = bcast(w1/sl)          (recip + PE transpose + DRAM-bounce bcast)
    L      = explid * bc           (DVE fp16 2x, in place)
    mid    = k_h.T q_h + L         (dots MM + identity MM accumulate in PSUM)
    E      = exp(mid)              (ACT, PSUM -> SBUF fp16)
    O      = [v_h | 1].T @ E       (PE; row 64 = softmax denominators s)
    rs     = 1/s                   (bounce s row to [8,128], exact recip)
    oT     = O[0:64] * bcast(rs)
    omT    = w_merge.T.T @ oT (+ b_merge)
  outT = w_out.T.T @ om (+ b_out) -> DMA out; host transposes back.

Softmax max-subtraction is dropped (|scores| <= ~20, exp safe) and conv_b
is dropped (softmax is shift-invariant along the reduced axis).
"""

import sys

try:
    import concourse.bass as bass
except ImportError:  # pragma: no cover
    sys.path.insert(0, "/opt/trn_rl_repo")
    import concourse.bass as bass

import numpy as np

import concourse.mybir as mybir
from concourse import bacc
from concourse.tile import TileContext
from concourse.bass_utils import run_bass_kernel_spmd

F32 = mybir.dt.float32
F16 = mybir.dt.float16
AX = mybir.AluOpType
EXP = mybir.ActivationFunctionType.Exp

B, N, DIM, H, DH = 8, 1024, 512, 8, 64
INNER = H * DH          # 512
QK = 2 * INNER          # 1024 (q|k feature rows of w_qkv)
SCALE = DH ** -0.5
LBIAS = -8.0            # lidar-exp shift: keeps exp(lsim) inside fp16 range
P = 128
NH = N // 2             # 512: max matmul free dim / fp32 PSUM bank
KC = DIM // P           # 4 contraction chunks
TC = N // P             # 8 token chunks
VW = DH + 1             # per-head v width incl. ones column

_cache = {}


def _build(w1, need_bm, need_bo):
    nc = bacc.Bacc("TRN2", target_bir_lowering=False, debug=False, num_devices=B)

    xT = nc.dram_tensor("xT", [DIM, N], F16, kind="ExternalInput")
    lidT = nc.dram_tensor("lidT", [DIM, N], F16, kind="ExternalInput")
    wqkT = nc.dram_tensor("wqkT", [DIM, QK], F16, kind="ExternalInput")
    wvT = nc.dram_tensor("wvT", [DIM, INNER], F16, kind="ExternalInput")
    wmT = nc.dram_tensor("wmT", [DH, DH], F16, kind="ExternalInput")
    woT = nc.dram_tensor("woT", [INNER, DIM], F16, kind="ExternalInput")
    ident = nc.dram_tensor("ident", [P, P], F16, kind="ExternalInput")
    identf = nc.dram_tensor("identf", [P, P], F32, kind="ExternalInput")
    onesv = nc.dram_tensor("onesv", [P, H, 1], F16, kind="ExternalInput")
    bm = nc.dram_tensor("bm", [DH, 1], F32, kind="ExternalInput")
    bo = nc.dram_tensor("bo", [P, KC], F32, kind="ExternalInput")
    y = nc.dram_tensor("y", [DIM, N], F32, kind="ExternalOutput")

    with TileContext(nc) as tc:
        with (
            tc.tile_pool(name="persist", bufs=1) as pp,
            tc.tile_pool(name="ps_w", bufs=3, space="PSUM") as ps_w,
            tc.tile_pool(name="ps_o", bufs=1, space="PSUM") as ps_o,
        ):
            # ---------------- persistent SBUF ----------------
            lid_sb = [pp.tile([P, N], F16, name=f"lid{i}", tag=f"lid{i}") for i in range(KC)]
            qT_sb = [pp.tile([P, N], F16, name=f"qT{i}", tag=f"qT{i}") for i in range(KC)]
            kT_sb = [pp.tile([P, N], F16, name=f"kT{i}", tag=f"kT{i}") for i in range(KC)]
            v_sb = [pp.tile([P, H * VW], F16, name=f"v{i}", tag=f"v{i}") for i in range(TC)]
            om_sb = [pp.tile([P, N], F16, name=f"om{i}", tag=f"om{i}") for i in range(KC)]
            id_sb = pp.tile([P, P], F16, name="ident", tag="ident")
            idf_sb = pp.tile([P, P], F32, name="identf", tag="identf")
            wm_sb = pp.tile([DH, DH], F16, name="wm", tag="wm")
            bm_sb = pp.tile([DH, 1], F32, name="bm", tag="bm")
            bo_sb = pp.tile([P, KC], F32, name="bo", tag="bo")
            lb_sb = pp.tile([P, 1], F32, name="lb", tag="lb")
            nc.vector.memset(lb_sb[:], LBIAS)
            wo_sb = [pp.tile([P, DIM], F16, name=f"wo{i}", tag=f"wo{i}") for i in range(KC)]
            for kc in range(KC):
                nc.sync.dma_start(wo_sb[kc][:], woT[kc * P:(kc + 1) * P, :])

            for c in range(KC):
                nc.sync.dma_start(lid_sb[c][:], lidT[c * P:(c + 1) * P, :])
            nc.sync.dma_start(id_sb[:], ident[:, :])
            nc.sync.dma_start(idf_sb[:], identf[:, :])
            nc.sync.dma_start(wm_sb[:], wmT[:, :])
            nc.sync.dma_start(bm_sb[:], bm[:, :])
            nc.sync.dma_start(bo_sb[:], bo[:, :])

            # phase 1 loads (the projection matmuls are interleaved into the
            # first pipeline iterations below to keep PE density high)
            lp = ctx_lp = tc.tile_pool(name="load", bufs=1)
            lp = ctx_lp.__enter__()
            x_sb = [lp.tile([P, N], F16, name=f"x{i}", tag=f"x{i}") for i in range(KC)]
            wqk_sb = [lp.tile([P, QK], F16, name=f"wqk{i}", tag=f"wqk{i}") for i in range(KC)]
            wv_sb = [lp.tile([P, INNER], F16, name=f"wv{i}", tag=f"wv{i}") for i in range(KC)]
            for c in range(KC):
                nc.sync.dma_start(x_sb[c][:], xT[c * P:(c + 1) * P, :])
                nc.sync.dma_start(wqk_sb[c][:], wqkT[c * P:(c + 1) * P, :])
                nc.sync.dma_start(wv_sb[c][:], wvT[c * P:(c + 1) * P, :])

            def emit_qk_group(fc):
                # qT|kT feature-major: out[fc,:] = sum_kc wqk[kc,fc].T @ xT[kc,:]
                dst = (qT_sb if fc < KC else kT_sb)[fc % KC]
                for ih in range(2):
                    pt = ps_w.tile([P, NH], F32, name="w", tag="w")
                    for kc in range(KC):
                        nc.tensor.matmul(
                            pt[:],
                            wqk_sb[kc][:, fc * P:(fc + 1) * P],
                            x_sb[kc][:, ih * NH:(ih + 1) * NH],
                            start=(kc == 0), stop=(kc == KC - 1),
                        )
                    nc.vector.tensor_copy(dst[:, ih * NH:(ih + 1) * NH], pt[:])

            def emit_v_group(t):
                # v token-major: v[t,:] = sum_kc xT[kc,t].T @ wvT[kc,:]
                pt = ps_w.tile([P, INNER], F32, name="w", tag="w")
                for kc in range(KC):
                    nc.tensor.matmul(
                        pt[:],
                        x_sb[kc][:, t * P:(t + 1) * P],
                        wv_sb[kc][:],
                        start=(kc == 0), stop=(kc == KC - 1),
                    )
                v3 = v_sb[t][:].rearrange("p (h w) -> p h w", h=H)
                nc.vector.tensor_copy(
                    v3[:, :, 0:DH], pt[:].rearrange("p (h d) -> p h d", h=H)
                )
                nc.sync.dma_start(v3[:, :, DH:VW], onesv[:, :, :])

            # ---------------- phase 2: per-head attention ----------------
            with (
                tc.tile_pool(name="el", bufs=16) as el_pool,
                tc.tile_pool(name="ework", bufs=4) as e_pool,
                tc.tile_pool(name="bc", bufs=3) as bc_pool,
                tc.tile_pool(name="small", bufs=3) as sm_pool,
                tc.tile_pool(name="dram", bufs=2, space="DRAM") as dr_pool,
            ):
                lid_hs = [lid_sb[h // 2][(h % 2) * DH:(h % 2) * DH + DH, :] for h in range(H)]
                q_hs = [qT_sb[h // 2][(h % 2) * DH:(h % 2) * DH + DH, :] for h in range(H)]
                k_hs = [kT_sb[h // 2][(h % 2) * DH:(h % 2) * DH + DH, :] for h in range(H)]
                st = {}
                ypar_pool = pp  # partial wout accumulators live in persist pool
                ypar_sb = [pp.tile([P, N], F32, name=f"ypar{i}", tag=f"ypar{i}") for i in range(KC)]
                QK_SCHED = {0: {1: 0, 5: 4}, 2: {1: 1, 5: 5}, 3: {3: 2},
                            4: {3: 6}, 5: {3: 3}, 6: {3: 7}}
                for it in range(H + 2):
                    # three-stage pipeline: lidar(hl) | attention(ha) | finish(hf)
                    hl, ha, hf = it, it - 1, it - 2
                    if hl < H:
                        st[hl] = {
                            "explid": [el_pool.tile([P, N], F16, name="explid", tag="explid") for _ in range(TC)],
                            "slc": sm_pool.tile([P, TC], F32, name="slc", tag="slc"),
                        }
                    if 0 <= hf:
                        # early O eviction (frees PSUM) + start the 1/s chain;
                        # results consumed at the end of this iteration
                        sf = st[hf]
                        rst = sm_pool.tile([VW, N], F32, name="rst", tag="rst")
                        nc.vector.tensor_copy(rst[DH:VW, :], sf["o"][DH:VW, :])
                        ot_un = bc_pool.tile([DH, N], F32, name="ot_un", tag="ot_un")
                        nc.vector.tensor_copy(ot_un[:], sf["o"][0:DH, :])
                        s_d = dr_pool.tile([1, N], F32, name="s_d", tag="s_d")
                        nc.sync.dma_start(s_d[:], rst[DH:VW, :])
                        s2 = sm_pool.tile([TC, P], F32, name="s2", tag="s2")
                        nc.sync.dma_start(
                            s2[:], s_d[:].rearrange("o (q p) -> (o q) p", p=P)
                        )
                        nc.vector.reciprocal(s2[:], s2[:])
                        rs_d = dr_pool.tile([1, N], F32, name="rs_d", tag="rs_d")
                        nc.sync.dma_start(
                            rs_d[:].rearrange("o (q p) -> (o q) p", p=P), s2[:]
                        )
                        brs_t = bc_pool.tile([DH, N], F32, name="brs", tag="brs")
                        nc.gpsimd.dma_start(brs_t[:], rs_d[:].to_broadcast((DH, N)))
                        sf["ot_un"], sf["brs"] = ot_un, brs_t
                    if 0 <= ha < H:
                        sa = st[ha]
                        sa["o"] = ps_o.tile([VW, N], F32, name="o", tag="o")
                        e_ts = {}
                    for jc in range(TC + 1):
                        # qk/v projection groups spread across iterations to
                        # keep PE density high (HAM stays warm); each group
                        # lands just in time for the head that needs it
                        fc = QK_SCHED.get(it, {}).get(jc)
                        if fc is not None:
                            emit_qk_group(fc)
                        if it == H + 1 and jc < TC and jc % 2 == 0:
                            # wout partials over om chunks 0..2 fill the drain
                            # iteration; chunk 3 lands after the last merge
                            yfc = jc // 2
                            pt = ps_w.tile([P, N], F32, name="w", tag="w")
                            for ih in range(2):
                                for kc in range(KC - 1):
                                    nc.tensor.matmul(
                                        pt[:, ih * NH:(ih + 1) * NH],
                                        wo_sb[kc][:, yfc * P:(yfc + 1) * P],
                                        om_sb[kc][:, ih * NH:(ih + 1) * NH],
                                        start=(kc == 0), stop=(kc == KC - 2),
                                    )
                            nc.vector.tensor_copy(ypar_sb[yfc][:], pt[:])
                        if it == 1 and jc < TC:
                            emit_v_group(jc)
                        if hl < H and jc < TC:
                            # lidar scores + biased exp w/ accumulate (sl rows
                            # via symmetry of lsim)
                            pt = ps_w.tile([P, N], F32, name="w", tag="w")
                            for ih in range(2):
                                nc.tensor.matmul(
                                    pt[:, ih * NH:(ih + 1) * NH],
                                    lid_hs[hl][:, jc * P:(jc + 1) * P],
                                    lid_hs[hl][:, ih * NH:(ih + 1) * NH],
                                    start=True, stop=True,
                                )
                            nc.scalar.activation(
                                st[hl]["explid"][jc][:], pt[:], EXP, bias=lb_sb[:],
                                accum_out=st[hl]["slc"][:, jc:jc + 1],
                            )
                        if 0 <= ha < H and jc < TC:
                            # L = explid * bc (in place); mid = dots + L; E
                            expl = sa["explid"]
                            nc.vector.tensor_mul(expl[jc][:], expl[jc][:], sa["bc"][:])
                            mid = ps_w.tile([P, N], F32, name="w", tag="w")
                            for ih in range(2):
                                nc.tensor.matmul(
                                    mid[:, ih * NH:(ih + 1) * NH],
                                    k_hs[ha][:, jc * P:(jc + 1) * P],
                                    q_hs[ha][:, ih * NH:(ih + 1) * NH],
                                    start=True, stop=False,
                                )
                                nc.tensor.matmul(
                                    mid[:, ih * NH:(ih + 1) * NH],
                                    id_sb[:],
                                    expl[jc][:, ih * NH:(ih + 1) * NH],
                                    start=False, stop=True,
                                )
                            e_t = e_pool.tile([P, N], F16, name="E", tag="E")
                            nc.scalar.activation(e_t[:], mid[:], EXP)
                            e_ts[jc] = e_t
                        if 0 <= ha < H and jc >= 1:
                            # vaug one step behind so PE never waits on exp
                            for ih in range(2):
                                nc.tensor.matmul(
                                    sa["o"][:, ih * NH:(ih + 1) * NH],
                                    v_sb[jc - 1][:, ha * VW:(ha + 1) * VW],
                                    e_ts[jc - 1][:, ih * NH:(ih + 1) * NH],
                                    start=(jc == 1), stop=(jc == TC),
                                )
                    if hl < H:
                        # w1/sl column -> PE transpose -> DRAM bounce -> bcast
                        slc = st[hl]["slc"]
                        nc.vector.reciprocal(slc[:], slc[:])
                        nc.vector.tensor_scalar(
                            out=slc[:], in0=slc[:], scalar1=float(w1),
                            scalar2=None, op0=AX.mult,
                        )
                        rt = sm_pool.tile([TC, P], F16, name="rt", tag="rt")
                        tr = ps_w.tile([P, NH], F32, name="w", tag="w")
                        nc.tensor.transpose(tr[0:TC, 0:P], slc[:], idf_sb[:])
                        nc.vector.tensor_copy(rt[:], tr[0:TC, 0:P])
                        rsl_d = dr_pool.tile([1, N], F16, name="rsl_d", tag="rsl_d")
                        nc.sync.dma_start(
                            rsl_d[:].rearrange("o (q p) -> (o q) p", p=P), rt[:]
                        )
                        bc_t = bc_pool.tile([P, N], F16, name="bc", tag="bc")
                        nc.sync.dma_start(
                            bc_t[0:64, :], rsl_d[:].to_broadcast((64, N))
                        )
                        nc.gpsimd.dma_start(
                            bc_t[64:P, :], rsl_d[:].to_broadcast((64, N))
                        )
                        st[hl]["bc"] = bc_t
                    if 0 <= hf:
                        # finish: normalize oT, merge, b_merge add
                        cf, offf = hf // 2, (hf % 2) * DH
                        ot_t = bc_pool.tile([DH, N], F16, name="ot", tag="ot")
                        nc.vector.tensor_mul(ot_t[:], sf["ot_un"][:], sf["brs"][:])
                        for ih in range(2):
                            mg = ps_w.tile([DH, NH], F32, name="w", tag="w")
                            nc.tensor.matmul(
                                mg[0:DH, 0:NH],
                                wm_sb[:],
                                ot_t[:, ih * NH:(ih + 1) * NH],
                                start=True, stop=True,
                            )
                            dst = om_sb[cf][offf:offf + DH, ih * NH:(ih + 1) * NH]
                            if need_bm:
                                nc.vector.tensor_scalar(
                                    out=dst, in0=mg[0:DH, 0:NH], scalar1=bm_sb[:],
                                    scalar2=None, op0=AX.add,
                                )
                            else:
                                nc.vector.tensor_copy(dst, mg[0:DH, 0:NH])
                        del st[hf]

            ctx_lp.__exit__(None, None, None)

            # ---------------- phase 3: final wout chunk + combine ----------------
            with tc.tile_pool(name="yout", bufs=2) as y_pool:
                for fc in range(KC):
                    pt = ps_w.tile([P, N], F32, name="w", tag="w")
                    for ih in range(2):
                        nc.tensor.matmul(
                            pt[:, ih * NH:(ih + 1) * NH],
                            wo_sb[KC - 1][:, fc * P:(fc + 1) * P],
                            om_sb[KC - 1][:, ih * NH:(ih + 1) * NH],
                            start=True, stop=True,
                        )
                    yt = y_pool.tile([P, N], F32, name="yt", tag="yt")
                    nc.vector.tensor_add(yt[:], pt[:], ypar_sb[fc][:])
                    if need_bo:
                        nc.vector.tensor_scalar(
                            out=yt[:], in0=yt[:], scalar1=bo_sb[:, fc:fc + 1],
                            scalar2=None, op0=AX.add,
                        )
                    nc.sync.dma_start(y[fc * P:(fc + 1) * P, :], yt[:])

    nc.compile()
    return nc


def kernel(x, lidar, w_qkv, w_merge, b_merge, w_out, b_out, conv_w, conv_b, **_):
    x = np.asarray(x, np.float32)
    lidar = np.asarray(lidar, np.float32)
    w_qkv = np.asarray(w_qkv, np.float32)
    w_merge = np.asarray(w_merge, np.float32)
    b_merge = np.asarray(b_merge, np.float32)
    w_out = np.asarray(w_out, np.float32)
    b_out = np.asarray(b_out, np.float32)
    w0, w1 = float(np.asarray(conv_w)[0]), float(np.asarray(conv_w)[1])

    need_bm = bool(np.any(b_merge != 0))
    need_bo = bool(np.any(b_out != 0))
    key = (round(w1, 12), need_bm, need_bo)
    if key not in _cache:
        _cache.clear()
        _cache[key] = _build(w1, need_bm, need_bo)
    nc = _cache[key]

    # host-side weight prep: transposes + constant folds + fp16 casts
    wqkT = np.ascontiguousarray(w_qkv[0:QK].T)       # [512 dim, 1024 q|k feats]
    wqkT[:, 0:INNER] *= np.float32(SCALE * w0)       # fold w0*SCALE into q
    wqkT = wqkT.astype(np.float16)
    wvT = np.ascontiguousarray(w_qkv[QK:3 * INNER].T).astype(np.float16)
    wmT = np.ascontiguousarray(w_merge.T).astype(np.float16)
    woT = np.ascontiguousarray(w_out.T).astype(np.float16)
    identity = np.eye(P, dtype=np.float16)
    identityf = np.eye(P, dtype=np.float32)
    bm_c = np.ascontiguousarray(b_merge.reshape(DH, 1))
    bo_c = np.ascontiguousarray(b_out.reshape(KC, P).T)

    sqrt_scale = np.float32(SCALE ** 0.5)
    in_maps = []
    for b in range(B):
        in_maps.append({
            "xT": np.ascontiguousarray(x[b].T).astype(np.float16),
            "lidT": (lidar[b].T * sqrt_scale).astype(np.float16),
            "wqkT": wqkT,
            "wvT": wvT,
            "wmT": wmT,
            "woT": woT,
            "ident": identity,
            "identf": identityf,
            "onesv": np.ones((P, H, 1), np.float16),
            "bm": bm_c,
            "bo": bo_c,
        })

    try:
        res = run_bass_kernel_spmd(nc, in_maps, core_ids=list(range(B)))
    except Exception:
        # transient NRT device wedges recover on a fresh attempt
        import time as _time

        _time.sleep(5)
        res = run_bass_kernel_spmd(nc, in_maps, core_ids=list(range(B)))
    kernel._last_results = res

    out = np.stack([res.results[b]["y"].T for b in range(B)])
    return (out, lidar)

